# revision 10
# baseline (speedup 1.0000x reference)
"""DigitCaps routing, batch-sharded across 8 TRN2 cores.

Each core owns 64 batch rows and the FULL capsule axis (I=1152), with W
replicated, so s/v/u_hat are entirely local per iteration. The only
cross-core data is the routing-logit update b_ij += mean_b <u_hat, v>,
a [1152, 10] f32 payload (46KB) all-reduced after iterations 1 and 2 --
vs the I-sharded baseline's three 336KB collectives. Matmul inputs are
bf16 (PE 4x faster than f32, tolerance 2e-2 is ample).

Per iteration: e = exp(b) broadcast (i16 -> (i16,j8) partitions) via an
indicator matmul; wc = W * e in g-chunks feeding the s-matmuls (72
accumulating MMs, contraction=128); squash folds the softmax denominator;
G = sum_b x (x) v per ij-tile (72 MMs, contraction=64); bupd = indicator
j-reduce of reduce_d(W * G).
"""
import numpy as np
import ml_dtypes

import concourse.bacc as bacc
import concourse.mybir as mybir
import concourse.tile as tile
from concourse.bass_utils import run_bass_kernel_spmd

N_CORES = 8
B, I, O, D, J = 512, 1152, 10, 16, 8
BL = B // N_CORES          # 64 local batch rows
IJ = I * J                 # 9216
G = IJ // 128              # 72 ij tiles of 128 partitions
OD = O * D                 # 160
GO = G * O                 # 720
GOH = GO // 2              # 360: max one PSUM bank per matmul output
NIT = 3
WCH = 12                   # g-chunk for wc formation (6 chunks)
GCH = 3                    # g-chunk for G-psum (24 chunks)
F32 = mybir.dt.float32
BF16 = mybir.dt.bfloat16
Act = mybir.ActivationFunctionType
Alu = mybir.AluOpType

_cache = {}


def _build(repeat=1, no_ar=False):
    nc = bacc.Bacc("TRN2", target_bir_lowering=False, debug=False,
                   num_devices=N_CORES)
    xT_e = nc.dram_tensor("xT", [IJ, BL], BF16, kind="ExternalInput")
    xN_e = nc.dram_tensor("xN", [BL, IJ], BF16, kind="ExternalInput")
    w2_e = nc.dram_tensor("w2", [IJ, OD], BF16, kind="ExternalInput")
    ind_e = nc.dram_tensor("ind", [16, 128], BF16, kind="ExternalInput")
    indj_e = nc.dram_tensor("indj", [128, 16], F32, kind="ExternalInput")
    v_e = nc.dram_tensor("v_out", [BL, OD], F32, kind="ExternalOutput")

    with tile.TileContext(nc) as tc:
        with (
            tc.tile_pool(name="const", bufs=1) as constp,
            tc.tile_pool(name="big", bufs=1) as big,
            tc.tile_pool(name="work", bufs=2) as work,
            tc.tile_pool(name="ps_g", bufs=2, space="PSUM") as ps_g_pool,
            tc.tile_pool(name="ps_s", bufs=2, space="PSUM") as ps_s_pool,
            tc.tile_pool(name="ps_e", bufs=2, space="PSUM") as ps_e_pool,
            tc.tile_pool(name="ps_m", bufs=2, space="PSUM") as ps_m_pool,
            tc.tile_pool(name="dram", bufs=2, space="DRAM") as dram,
        ):
            # ---- persistent inputs ----
            xT = big.tile([128, G * BL], BF16)      # [p=ij%128, (g, b)]
            w2 = big.tile([128, G * OD], BF16)      # [p=ij%128, (g, o, d)]
            xN = big.tile([BL, IJ], BF16)           # [p=b, ij]
            for g in range(G):
                nc.sync.dma_start(out=w2[:, g * OD:(g + 1) * OD],
                                  in_=w2_e[g * 128:(g + 1) * 128, :])
                nc.sync.dma_start(out=xT[:, g * BL:(g + 1) * BL],
                                  in_=xT_e[g * 128:(g + 1) * 128, :])
            nc.sync.dma_start(out=xN[:, :], in_=xN_e[:, :])
            ind = constp.tile([16, 128], BF16)
            nc.sync.dma_start(out=ind[:], in_=ind_e[:])
            indj = constp.tile([128, 16], F32)
            nc.sync.dma_start(out=indj[:], in_=indj_e[:])
            ones16 = constp.tile([16, 1], BF16)
            nc.vector.memset(ones16[:], 1.0)
            ones64 = constp.tile([1, 64], F32)
            nc.vector.memset(ones64[:], 1.0)

            IVD0 = 1.0 / I                          # uniform softmax at t=0

            def emit_stage(t, st):
                # Stages are emitted software-pipelined across reps (see
                # driver below) so each collective's latency window holds
                # independent compute from an adjacent rep's stage.
                if True:
                    last = t == NIT - 1
                    if t == 0:
                        s_rhs = w2
                    else:
                        # e = exp(b) [16, GO] bf16; den[o] = sum_i e
                        e_c = work.tile([16, GO], BF16, name="e_c")
                        nc.scalar.activation(e_c[:], st["b"][:], Act.Exp)
                        den = work.tile([1, 32], F32, name="den")
                        dh = work.tile([1, 64], F32, name="dh")
                        for h in range(2):
                            hs = slice(h * GOH, (h + 1) * GOH)
                            ps_dh = ps_m_pool.tile([1, GOH], F32,
                                                   name="ps_dh",
                                                   tag="ps_misc")
                            nc.tensor.matmul(ps_dh[:], ones16[:], e_c[:, hs],
                                             start=True, stop=True)
                            nc.vector.reduce_sum(
                                dh[0:1, h * 16:h * 16 + O],
                                ps_dh[:].rearrange("p (g o) -> p o g",
                                                   g=G // 2),
                                axis=mybir.AxisListType.X)
                        nc.vector.tensor_tensor(den[0:1, 0:O],
                                                dh[0:1, 0:O],
                                                dh[0:1, 16:16 + O],
                                                op=Alu.add)
                        ivq = work.tile([1, 32], F32, name="ivq")
                        nc.vector.reciprocal(ivq[0:1, 0:O], den[0:1, 0:O])
                        nc.vector.tensor_tensor(ivq[0:1, 16:16 + O],
                                                ivq[0:1, 0:O], ivq[0:1, 0:O],
                                                op=Alu.mult)
                        # broadcast e across j-partitions: [16,GO] -> [128,GO]
                        e_bc = work.tile([128, GO], BF16, name="e_bc")
                        for h in range(2):
                            hs = slice(h * GOH, (h + 1) * GOH)
                            ps_e = ps_e_pool.tile([128, GOH], F32,
                                                  name="ps_e")
                            nc.tensor.matmul(ps_e[:], ind[:], e_c[:, hs],
                                             start=True, stop=True)
                            nc.scalar.activation(e_bc[:, hs], ps_e[:],
                                                 Act.Copy)
                        # wc = w2 * e (broadcast over d), chunked by g
                        wc = work.tile([128, G * OD], BF16, name="wc", bufs=1)
                        for c in range(G // WCH):
                            od_s = slice(c * WCH * OD, (c + 1) * WCH * OD)
                            o_s = slice(c * WCH * O, (c + 1) * WCH * O)
                            nc.vector.tensor_tensor(
                                wc[:, od_s].rearrange(
                                    "p (g o d) -> p g o d", g=WCH, o=O),
                                w2[:, od_s].rearrange(
                                    "p (g o d) -> p g o d", g=WCH, o=O),
                                e_bc[:, o_s].rearrange(
                                    "p (g o) -> p g o", g=WCH)
                                .unsqueeze(3).broadcast_to([128, WCH, O, D]),
                                op=Alu.mult)
                        s_rhs = wc

                    # s[b, od] accumulated over all 72 ij tiles;
                    # cols 160:192 hold the ivd/ivd^2 broadcast (t>0)
                    ps_s = ps_s_pool.tile([64, OD + 32], F32, name="ps_s")
                    if t > 0:
                        nc.tensor.matmul(ps_s[:, OD:OD + 32], ones64[:],
                                         ivq[:], start=True, stop=True)
                    for g in range(G):
                        nc.tensor.matmul(
                            ps_s[:, 0:OD], xT[:, g * BL:(g + 1) * BL],
                            s_rhs[:, g * OD:(g + 1) * OD],
                            start=(g == 0), stop=(g == G - 1))

                    # squash with folded softmax normalization
                    sqr = work.tile([64, OD], F32, name="sqr")
                    nc.scalar.activation(sqr[:], ps_s[:, 0:OD], Act.Square)
                    sqs = work.tile([64, O], F32, name="sqs")
                    nc.vector.reduce_sum(
                        sqs[:], sqr[:].rearrange("p (o d) -> p o d", o=O),
                        axis=mybir.AxisListType.X)
                    sqt = work.tile([64, O], F32, name="sqt")
                    if t == 0:
                        nc.vector.tensor_scalar_mul(sqt[:], sqs[:],
                                                    IVD0 * IVD0)
                    else:
                        nc.vector.tensor_tensor(sqt[:], sqs[:],
                                                ps_s[:, OD + 16:OD + 16 + O],
                                                op=Alu.mult)
                    rt = work.tile([64, O], F32, name="rt")
                    nc.scalar.activation(rt[:], sqt[:], Act.Sqrt)
                    d2 = work.tile([64, O], F32, name="d2")
                    nc.vector.tensor_scalar_add(d2[:], sqt[:], 1.0)
                    rc = work.tile([64, O], F32, name="rc")
                    nc.vector.reciprocal(rc[:], d2[:])
                    gf = work.tile([64, O], F32, name="gf")
                    nc.vector.tensor_tensor(gf[:], rt[:], rc[:], op=Alu.mult)
                    gf2 = work.tile([64, O], F32, name="gf2")
                    if t == 0:
                        nc.vector.tensor_scalar_mul(gf2[:], gf[:], IVD0)
                    else:
                        nc.vector.tensor_tensor(gf2[:], gf[:],
                                                ps_s[:, OD:OD + O], op=Alu.mult)
                    if last:
                        v_sl = work.tile([64, OD], F32, name="v_sl")
                        nc.vector.tensor_tensor(
                            v_sl[:].rearrange("p (o d) -> p o d", o=O),
                            ps_s[:, 0:OD].rearrange("p (o d) -> p o d", o=O),
                            gf2[:].unsqueeze(2).broadcast_to([64, O, D]),
                            op=Alu.mult)
                        nc.sync.dma_start(out=v_e[:, :], in_=v_sl[:])
                        return
                    v_bf = work.tile([64, OD], BF16, name="v_bf")
                    nc.vector.tensor_tensor(
                        v_bf[:].rearrange("p (o d) -> p o d", o=O),
                        ps_s[:, 0:OD].rearrange("p (o d) -> p o d", o=O),
                        gf2[:].unsqueeze(2).broadcast_to([64, O, D]),
                        op=Alu.mult)

                    # G = sum_b x (x) v per ij tile; p4d = reduce_d(w2 * G)
                    p4d = work.tile([128, GO], F32, name="p4d")
                    for c in range(G // GCH):
                        ps_g = ps_g_pool.tile([128, GCH * OD], F32,
                                              name="ps_gc")
                        for k in range(GCH):
                            g = c * GCH + k
                            nc.tensor.matmul(
                                ps_g[:, k * OD:(k + 1) * OD],
                                xN[:, g * 128:(g + 1) * 128], v_bf[:],
                                start=True, stop=True)
                        p4t = work.tile([128, GCH * OD], BF16, name="p4t")
                        od_s = slice(c * GCH * OD, (c + 1) * GCH * OD)
                        nc.vector.tensor_tensor(
                            p4t[:], w2[:, od_s], ps_g[:], op=Alu.mult)
                        nc.vector.reduce_sum(
                            p4d[:, c * GCH * O:(c + 1) * GCH * O],
                            p4t[:].rearrange("p (g o d) -> p g o d",
                                             g=GCH, o=O),
                            axis=mybir.AxisListType.X)
                    bupd = work.tile([16, GO], F32, name="bupd")
                    for h in range(2):
                        hs = slice(h * GOH, (h + 1) * GOH)
                        ps_b = ps_m_pool.tile([16, GOH], F32, name="ps_b",
                                              tag="ps_misc")
                        nc.tensor.matmul(ps_b[:], indj[:], p4d[:, hs],
                                         start=True, stop=True)
                        nc.vector.tensor_copy(bupd[:, hs], ps_b[:])

                    ar_in = dram.tile([16, GO], F32, name="ar_in")
                    ar_out = dram.tile([16, GO], F32, name="ar_out")
                    nc.sync.dma_start(out=ar_in[:, :], in_=bupd[:])
                    if no_ar:
                        nc.sync.dma_start(out=ar_out[:, :], in_=ar_in[:, :])
                    else:
                        nc.gpsimd.collective_compute(
                            "AllReduce", Alu.add,
                            replica_groups=[list(range(N_CORES))],
                            ins=[ar_in.opt()], outs=[ar_out.opt()])
                    upd = work.tile([16, GO], F32, name="upd")
                    nc.sync.dma_start(out=upd[:], in_=ar_out[:, :])
                    if t == 0:
                        st["b"] = upd
                    else:
                        b_new = work.tile([16, GO], F32, name="b_new")
                        nc.vector.tensor_tensor(b_new[:], st["b"][:], upd[:],
                                                op=Alu.add)
                        st["b"] = b_new

            # t0 of rep r+1 is independent of rep r's collectives, so emit
            # it between t1(r) and t2(r): it fills AR1(r)'s latency window
            # and has its own AR0(r+1) queued right behind AR1(r).
            states = [{} for _ in range(repeat)]
            emit_stage(0, states[0])
            for rep in range(repeat):
                emit_stage(1, states[rep])
                if rep + 1 < repeat:
                    emit_stage(0, states[rep + 1])
                emit_stage(2, states[rep])

    nc.compile()
    return nc


def _host_inputs(x, W):
    x = np.ascontiguousarray(x, dtype=np.float32)
    W = np.ascontiguousarray(W, dtype=np.float32)
    bf = ml_dtypes.bfloat16
    w2 = np.ascontiguousarray(
        W.transpose(0, 3, 1, 2).reshape(IJ, OD)).astype(bf)
    ind = np.zeros((16, 128), dtype=np.float32)
    for k in range(16):
        ind[k, k * 8:(k + 1) * 8] = 1.0
    indb = ind.astype(bf)
    indj = np.ascontiguousarray(ind.T / float(B)).astype(np.float32)
    in_maps = []
    for c in range(N_CORES):
        xs = x[c * BL:(c + 1) * BL].reshape(BL, IJ)
        in_maps.append({
            "xT": np.ascontiguousarray(xs.T).astype(bf),
            "xN": xs.astype(bf),
            "w2": w2,
            "ind": indb,
            "indj": indj,
        })
    return in_maps


def kernel(x, W):
    if "nc" not in _cache:
        _cache["nc"] = _build()
    nc = _cache["nc"]
    in_maps = _host_inputs(x, W)
    res = run_bass_kernel_spmd(nc, in_maps, list(range(N_CORES)))
    v = np.concatenate([res.results[k]["v_out"] for k in range(N_CORES)],
                       axis=0)
    return v.reshape(B, O, D, 1).astype(np.float32)
